# revision 5
# baseline (speedup 1.0000x reference)
"""Trainium2 Bass kernel for nn_AdjAttenAggr (masked attention aggregation).

Math (reference):
    Q = main_feat @ Wq.T + bq                 [N, MID]
    K = other_feat @ Wk.T + bk                [M, MID]
    A = softmax(where(mask, -2^32, Q K^T / sqrt(MID)), axis=-1)
    out = A @ (fix_feat[:, None] * other_feat)   [N, KDIM]

Strategy: row-parallel across 8 NeuronCores (1024 rows of N each), K/V
replicated.  On each core, flash-style: S^T tiles [m=128, q] from PE
(bf16 operands, fp32 PSUM), exp on ACT (scale folded in; scores are
small so no max subtraction is needed), multiplicative mask on DVE
(exp(s+NEG*mask) == exp(s)*(1-mask) since exp(-2^32) == 0), then
P^T @ V on PE accumulating over m.  Row sums ride along as a
ones-stationary matmul; normalization happens after the AV matmul since
softmax(S) @ V == diag(1/rowsum) @ (exp(S) @ V).
"""

import numpy as np
import ml_dtypes

import concourse.bacc as bacc
import concourse.tile as tile
from concourse import mybir
from concourse.bass_utils import run_bass_kernel_spmd

BF16 = ml_dtypes.bfloat16

N, M = 8192, 8192
QDIM, KDIM, MID = 512, 512, 256
SCALE = float(np.sqrt(MID))
NCORES = 8
NSH = N // NCORES          # 1024 rows per core
P = 128                    # partitions
N_MT = M // P              # 64 m-tiles
N_ST = NSH // 512          # 2 supertiles of 512 q rows
N_JT = QDIM // P           # 4 contraction tiles for projections
N_DT = MID // P            # 2 d-tiles

_BUILT = None


def build_nc():
    nc = bacc.Bacc(None, target_bir_lowering=False, debug=False)
    f32 = mybir.dt.float32
    bf = mybir.dt.bfloat16

    mainT = nc.declare_dram_parameter("mainT", [QDIM, NSH], bf, isOutput=False)
    otherT = nc.declare_dram_parameter("otherT", [KDIM, M], bf, isOutput=False)
    other = nc.declare_dram_parameter("other", [M, KDIM], bf, isOutput=False)
    nmaskT = nc.declare_dram_parameter("nmaskT", [M, NSH], bf, isOutput=False)
    wqT = nc.declare_dram_parameter("wqT", [QDIM, MID], bf, isOutput=False)
    wkT = nc.declare_dram_parameter("wkT", [KDIM, MID], bf, isOutput=False)
    bq_t = nc.declare_dram_parameter("bq_t", [P, N_DT], f32, isOutput=False)
    bk_t = nc.declare_dram_parameter("bk_t", [P, N_DT], f32, isOutput=False)
    fix_t = nc.declare_dram_parameter("fix_t", [P, N_MT], f32, isOutput=False)
    out = nc.declare_dram_parameter("out", [NSH, KDIM], f32, isOutput=True)

    with tile.TileContext(nc) as tc:
        with (
            tc.tile_pool(name="const", bufs=1) as const,
            tc.tile_pool(name="wstream", bufs=8) as wstream,
            tc.tile_pool(name="vstream", bufs=4) as vstream,
            tc.tile_pool(name="mstream", bufs=6) as mstream,
            tc.tile_pool(name="ptiles", bufs=4) as ptiles,
            tc.tile_pool(name="pmtiles", bufs=4) as pmtiles,
            tc.tile_pool(name="otiles", bufs=4) as otiles,
            tc.tile_pool(name="small", bufs=4) as small,
            tc.tile_pool(name="ps_s", bufs=2, space="PSUM") as ps_s,
            tc.tile_pool(name="ps_av", bufs=1, space="PSUM") as ps_av,
            tc.tile_pool(name="ps_sum", bufs=1, space="PSUM") as ps_sum,
        ):
            # ---- persistent SBUF ----
            wq_sb = const.tile([P, N_JT, MID], bf)    # WqT j-tiles
            wk_sb = const.tile([P, N_JT, MID], bf)
            bq_sb = const.tile([P, N_DT], f32)
            bk_sb = const.tile([P, N_DT], f32)
            fix_sb = const.tile([P, N_MT], f32)
            ones_sb = const.tile([P, 1], bf)
            kt_sb = const.tile([P, N_DT, M], bf)      # K^T, d-tiles x m
            qt_sb = const.tile([P, N_DT, NSH], bf)    # Q^T, d-tiles x q
            v_sb = const.tile([P, N_MT, KDIM], bf)    # V m-tiles

            nc.sync.dma_start(out=wq_sb[:], in_=wqT.rearrange("(j p) d -> p j d", p=P))
            nc.sync.dma_start(out=wk_sb[:], in_=wkT.rearrange("(j p) d -> p j d", p=P))
            nc.sync.dma_start(out=bq_sb[:], in_=bq_t[:])
            nc.sync.dma_start(out=bk_sb[:], in_=bk_t[:])
            nc.sync.dma_start(out=fix_sb[:], in_=fix_t[:])
            nc.vector.memset(ones_sb[:], 1.0)

            # ---- V = fix * other, bf16, m on partitions ----
            for mt in range(N_MT):
                o_t = vstream.tile([P, KDIM], bf, tag="vsrc")
                nc.sync.dma_start(out=o_t[:], in_=other[mt * P:(mt + 1) * P, :])
                nc.vector.tensor_scalar_mul(
                    v_sb[:, mt, :], o_t[:], fix_sb[:, mt:mt + 1]
                )

            # ---- K^T = Wk @ other^T + bk (d on partitions) ----
            MC = 512
            for mc in range(M // MC):
                ot_tiles = []
                for j in range(N_JT):
                    t = wstream.tile([P, MC], bf, tag="otherT")
                    nc.sync.dma_start(
                        out=t[:],
                        in_=otherT[j * P:(j + 1) * P, mc * MC:(mc + 1) * MC],
                    )
                    ot_tiles.append(t)
                for d in range(N_DT):
                    ps = ps_s.tile([P, MC], f32, tag="s")
                    for j in range(N_JT):
                        nc.tensor.matmul(
                            ps[:],
                            wk_sb[:, j, d * P:(d + 1) * P],
                            ot_tiles[j][:],
                            start=(j == 0),
                            stop=(j == N_JT - 1),
                        )
                    nc.scalar.activation(
                        kt_sb[:, d, mc * MC:(mc + 1) * MC], ps[:],
                        mybir.ActivationFunctionType.Identity,
                        bias=bk_sb[:, d:d + 1],
                    )

            # ---- Q^T = Wq @ main^T + bq ----
            mt_tiles = []
            for j in range(N_JT):
                t = wstream.tile([P, NSH], bf, tag="mainT")
                nc.sync.dma_start(out=t[:], in_=mainT[j * P:(j + 1) * P, :])
                mt_tiles.append(t)
            for d in range(N_DT):
                for qc in range(NSH // 512):
                    ps = ps_s.tile([P, 512], f32, tag="s")
                    for j in range(N_JT):
                        nc.tensor.matmul(
                            ps[:],
                            wq_sb[:, j, d * P:(d + 1) * P],
                            mt_tiles[j][:, qc * 512:(qc + 1) * 512],
                            start=(j == 0),
                            stop=(j == N_JT - 1),
                        )
                    nc.scalar.activation(
                        qt_sb[:, d, qc * 512:(qc + 1) * 512], ps[:],
                        mybir.ActivationFunctionType.Identity,
                        bias=bq_sb[:, d:d + 1],
                    )

            # ---- main attention loop ----
            for st in range(N_ST):
                q0 = st * 512
                av = ps_av.tile([P, 4, KDIM], f32, tag="av")
                sums = ps_sum.tile([P, 4], f32, tag="sums")
                for mt in range(N_MT):
                    s_ps = ps_s.tile([P, 512], f32, tag="s")
                    for d in range(N_DT):
                        nc.tensor.matmul(
                            s_ps[:],
                            kt_sb[:, d, mt * P:(mt + 1) * P],
                            qt_sb[:, d, q0:q0 + 512],
                            start=(d == 0),
                            stop=(d == N_DT - 1),
                        )
                    p_t = ptiles.tile([P, 512], bf, tag="p")
                    nc.scalar.activation(
                        p_t[:], s_ps[:],
                        mybir.ActivationFunctionType.Exp,
                        scale=1.0 / SCALE,
                    )
                    nm_t = mstream.tile([P, 512], bf, tag="nm")
                    nc.sync.dma_start(
                        out=nm_t[:],
                        in_=nmaskT[mt * P:(mt + 1) * P, q0:q0 + 512],
                    )
                    pm_t = pmtiles.tile([P, 512], bf, tag="pm")
                    nc.vector.tensor_mul(pm_t[:], p_t[:], nm_t[:])
                    for qs in range(4):
                        nc.tensor.matmul(
                            av[:, qs, :],
                            pm_t[:, qs * P:(qs + 1) * P],
                            v_sb[:, mt, :],
                            start=(mt == 0),
                            stop=(mt == N_MT - 1),
                        )
                        # row sums land per-partition: P^T_qs.T @ ones.
                        # All four qs columns share one 2KB PSUM bank =
                        # one zero region, so exactly one start/stop pair
                        # for the whole bank.
                        nc.tensor.matmul(
                            sums[:, qs:qs + 1],
                            pm_t[:, qs * P:(qs + 1) * P],
                            ones_sb[:, 0:1],
                            start=(mt == 0 and qs == 0),
                            stop=(mt == N_MT - 1 and qs == 3),
                        )
                recip = small.tile([P, 4], f32, tag="recip")
                nc.vector.reciprocal(recip[:], sums[:])
                for qs in range(4):
                    o_sb = otiles.tile([P, KDIM], f32, tag="o")
                    nc.scalar.activation(
                        o_sb[:], av[:, qs, :],
                        mybir.ActivationFunctionType.Copy,
                        scale=recip[:, qs:qs + 1],
                    )
                    nc.sync.dma_start(
                        out=out[q0 + qs * P:q0 + (qs + 1) * P, :],
                        in_=o_sb[:],
                    )

    nc.compile()
    return nc


def _get_nc():
    global _BUILT
    if _BUILT is None:
        _BUILT = build_nc()
    return _BUILT


def kernel(main_feat, other_feat, fix_feat, mask, Wq, bq, Wk, bk):
    main_feat = np.asarray(main_feat, dtype=np.float32)
    other_feat = np.asarray(other_feat, dtype=np.float32)
    fix_feat = np.asarray(fix_feat, dtype=np.float32)
    mask = np.asarray(mask)
    Wq = np.asarray(Wq, dtype=np.float32)
    bq = np.asarray(bq, dtype=np.float32)
    Wk = np.asarray(Wk, dtype=np.float32)
    bk = np.asarray(bk, dtype=np.float32)

    nc = _get_nc()

    otherT_bf = np.ascontiguousarray(other_feat.T).astype(BF16)
    other_bf = other_feat.astype(BF16)
    wqT_bf = np.ascontiguousarray(Wq.T).astype(BF16)
    wkT_bf = np.ascontiguousarray(Wk.T).astype(BF16)
    bq_t = np.ascontiguousarray(bq.reshape(N_DT, P).T)
    bk_t = np.ascontiguousarray(bk.reshape(N_DT, P).T)
    fix_t = np.ascontiguousarray(fix_feat.reshape(N_MT, P).T)
    # notmask, transposed: [M, N] in bf16 (exact 0.0 / 1.0)
    nmaskT_bf = np.ascontiguousarray((~mask).T).astype(BF16)

    in_maps = []
    for c in range(NCORES):
        r0, r1 = c * NSH, (c + 1) * NSH
        in_maps.append({
            "mainT": np.ascontiguousarray(main_feat[r0:r1].T).astype(BF16),
            "otherT": otherT_bf,
            "other": other_bf,
            "nmaskT": np.ascontiguousarray(nmaskT_bf[:, r0:r1]),
            "wqT": wqT_bf,
            "wkT": wkT_bf,
            "bq_t": bq_t,
            "bk_t": bk_t,
            "fix_t": fix_t,
        })

    res = run_bass_kernel_spmd(nc, in_maps, list(range(NCORES)))
    return np.concatenate([res.results[c]["out"] for c in range(NCORES)], axis=0)
